# revision 1
# baseline (speedup 1.0000x reference)
"""BankModulatedConv Trainium2 kernel.

Problem (per sample b of B=8, one NeuronCore per sample):
  w = softmax(bank_request[b])                        # (16,)
  kern = sum_f w[f] * bank_weight[f]                  # (o, i, kh, kw) = (256, 256, 3, 3)
  kern *= (1 + style[b, i])                           # input-channel modulation
  kern *= rsqrt(sum_{i,kh,kw} kern^2 + 1e-8)          # per-o L2 demodulation
  y[b] = conv2d(x[b], kern, stride 1, SAME)           # (256, 64, 64)

Mapping (data-parallel over batch; all math on device):
  - The filter bank ships host-rearranged to
      [oc(2), ic(2), fq(4), i(128), f(4), o_local(128), khw(9)]  (bf16)
    so each DMA row is >= 9216 contiguous bytes (fat descriptors; thin
    column DMAs cost ~120ns/descriptor and wreck the pipeline start),
    o-chunk-major so conv(oc0) overlaps the oc1 DMA, and the mixed kernel
    lands directly in conv lhsT layout [i, (o,khw)] with no transposes.
    bf16 is storage precision only -- mixing accumulates in fp32.
    The first tile carries 257 extra constant columns (I_128, ones) so
    constants arrive on fat rows too.
  - Mixing: 3 of 4 (oc, ic) blocks on TensorE (lhsT_f = w[f] * I_128,
    16 accumulated bf16 matmuls per psum slice); the (oc1, ic0) block
    runs as a scalar_tensor_tensor MAC chain on VectorE to shorten the
    TensorE critical path. DMA issue order + per-block pool tags pace
    arrivals to match consumption.
  - style modulation = per-partition scalar (1+style[i]) fused into the
    PSUM->SBUF copy; softmax weights / style / demod scales are spread
    across partitions with tiny K=1 matmuls (never column DMAs).
  - demod: square + reduce-over-khw on DVE, then a ones-vector matmul
    reduces across the i partition dim; the rsqrt'd scale is applied per
    output channel in the ScalarE conv-PSUM-out copy.
  - conv: per (o_chunk, 8-row spatial tile): 18 accumulated float32r
    matmuls (i_chunk x 3 x 3) over a host-pre-padded x tile in SBUF.
"""
import sys

if "/opt/trn_rl_repo" not in sys.path:
    sys.path.insert(0, "/opt/trn_rl_repo")

import numpy as np
import concourse.bacc as bacc
import concourse.mybir as mybir
import concourse.tile as tile
from concourse.alu_op_type import AluOpType
from concourse.bass_utils import run_bass_kernel_spmd

dt = mybir.dt
AF = mybir.ActivationFunctionType

B, F, D, KK, H, W = 8, 16, 256, 3, 64, 64
HW = H * W            # 4096
KHW = KK * KK         # 9
IC = D // 128         # 2 i-chunks
OC = D // 128         # 2 o-chunks
FQ = 4                # f quarters per bank DMA tile
FPQ = F // FQ         # 4 f per quarter
OCK = 128 * KHW       # 1152 free elems per (o_chunk, khw) group
BROW = FPQ * OCK      # 4608 bf16 elems per (oc, ic, fq, i) DMA row
PW = W + 2            # padded width 66
PH_ = H + 2           # padded height 66
NS = 8                # spatial tiles (8 rows each)
SROWS = H // NS       # 8 rows per spatial tile
SN = SROWS * W        # 512 = conv matmul moving size

_COMPILED = None


def _build(num_devices=B):
    nc = bacc.Bacc("TRN2", target_bir_lowering=False, debug=False,
                   num_devices=num_devices)

    x_d = nc.dram_tensor("x", [D, PH_ * PW], dt.float32, kind="ExternalInput").ap()
    # first 128 rows carry BROW+257 columns: the trailing 257 are constants
    # (I_128, ones column, ones row on partition 0) -- embedded here so they
    # arrive via fat contiguous rows instead of a 128-descriptor column DMA
    bank_d = nc.dram_tensor("bank", [OC * IC * FQ * 128, BROW + 257], dt.bfloat16,
                            kind="ExternalInput").ap()
    breq_d = nc.dram_tensor("breq", [1, F], dt.float32, kind="ExternalInput").ap()
    sty_d = nc.dram_tensor("sty", [1, D], dt.float32, kind="ExternalInput").ap()
    y_d = nc.dram_tensor("y", [D, HW], dt.float32, kind="ExternalOutput").ap()

    f32, f32r, bf16 = dt.float32, dt.float32r, dt.bfloat16

    with tile.TileContext(nc) as tc:
        with (
            tc.tile_pool(name="setup", bufs=1) as setup,
            tc.tile_pool(name="xp", bufs=1) as xp,
            tc.tile_pool(name="bankp", bufs=3) as bankp,
            tc.tile_pool(name="kern", bufs=1) as kernp,
            tc.tile_pool(name="yout", bufs=4) as youtp,
            tc.tile_pool(name="dram", bufs=1, space="DRAM") as dramp,
            tc.tile_pool(name="mixps", bufs=1, space="PSUM") as mixps,
            tc.tile_pool(name="convps", bufs=3, space="PSUM") as convps,
            tc.tile_pool(name="normps", bufs=1, space="PSUM") as normps,
        ):
            # tiny control DMAs first so they land before the bank megabytes
            breq = setup.tile([1, F], dt.float32)
            nc.sync.dma_start(breq[:], breq_d[:])
            styrow = setup.tile([1, D], dt.float32)
            nc.sync.dma_start(styrow[:], sty_d[:])

            # ---------- bank DMAs, in consumption order (oc0 first) ----------
            # The (oc1, ic0) block is mixed by the DVE MAC chain, which reads
            # its tiles slowly -- give those a separate slot tag and order
            # them last so they can't stall the PE block's DMAs.
            bts = {}

            def issue_bank_dmas(oc, order):
                for ic, fq in order:
                    tag = {(1, 0): "bankmac", (1, 1): "bank1"}.get((oc, ic), "bank0")
                    wide = oc == 0 and ic == 0 and fq == 0
                    cols = BROW + 257 if wide else BROW
                    bt = bankp.tile([128, cols], bf16, tag=tag)
                    row0 = ((oc * IC + ic) * FQ + fq) * 128
                    nc.sync.dma_start(bt[:], bank_d[row0:row0 + 128, 0:cols])
                    bts[(oc, ic, fq)] = bt

            # oc0: slot-paced (bufs=3) so quarters arrive in consumption order
            issue_bank_dmas(0, [(0, 0), (0, 1), (0, 2), (0, 3), (1, 0), (1, 1)])

            # ---------- x: host-pre-padded, straight DMA ----------
            # Issued two quarters before the oc0 stream ends: conv(oc0) start
            # is gated by max(mix(oc0) end, x arrival) -- this balances them.
            xpads = []
            for ic in range(IC):
                xpad = xp.tile([128, PH_ * PW], f32r, tag=f"xpad{ic}")
                nc.sync.dma_start(
                    xpad[:], x_d[ic * 128:(ic + 1) * 128, :].bitcast(f32r))
                xpads.append(xpad)

            issue_bank_dmas(0, [(1, 2), (1, 3)])
            # oc1: MAC tiles q0/q1 lead (the DVE chain is the long pole), then
            # the PE-block tiles; all fill the DMA-idle conv(oc0) window
            issue_bank_dmas(1, [(0, 0), (0, 1), (1, 0), (1, 1), (1, 2), (1, 3),
                                (0, 2), (0, 3)])

            # ---------- setup: softmax weights, diag tiles, style columns ----------
            bt0 = bts[(0, 0, 0)]
            ident = bt0[:, BROW:BROW + 128]            # I_128 (bf16-exact)
            onescol = bt0[:, BROW + 128:BROW + 129]

            # softmax without the max-shift: inputs are O(1), and f32 exp
            # only overflows past ~88, so the shift is dead weight on the
            # critical path to the first mix matmul.
            ex = setup.tile([1, F], f32)
            nc.scalar.activation(ex[:], breq[:], AF.Exp, bias=0.0, scale=1.0)
            sm = setup.tile([1, 1], f32)
            nc.vector.reduce_sum(sm[:], ex[:], axis=mybir.AxisListType.X)
            rs = setup.tile([1, 1], f32)
            nc.vector.reciprocal(rs[:], sm[:])
            wrow = setup.tile([1, F], f32)
            nc.vector.tensor_scalar(out=wrow[:], in0=ex[:], scalar1=rs[:],
                                    scalar2=None, op0=AluOpType.mult)
            # broadcast w across partitions with a K=1 bf16 matmul
            # (gpsimd partition_broadcast stalls ~9us on a ucode reload whose
            # fetch DMA sits behind the bank megabytes)
            wrow_b = setup.tile([1, F], bf16)
            with nc.allow_low_precision(reason="broadcast only; values tiny-rank"):
                nc.vector.tensor_copy(wrow_b[:], wrow[:])
            onesrow_b = bt0[0:1, BROW + 129:BROW + 257]
            wbps = normps.tile([128, F], f32, tag="aux")
            nc.tensor.matmul(wbps[:], onesrow_b[:], wrow_b[:], start=True, stop=True)
            wbc = setup.tile([128, F], f32)
            nc.vector.tensor_copy(wbc[:], wbps[:])

            # per-f diagonal lhsT tiles diag(w_f), bf16 for the mix matmuls
            diags = []
            with nc.allow_low_precision(reason="bf16 diag weights; mix accumulates f32"):
                for f in range(F):
                    dg = setup.tile([128, 128], bf16, tag=f"diag{f}")
                    nc.vector.tensor_scalar(out=dg[:], in0=ident[:],
                                            scalar1=wbc[:, f:f + 1],
                                            scalar2=None, op0=AluOpType.mult)
                    diags.append(dg)

            # style columns (1 + style[i]) as per-partition scalars, per i-chunk
            # (1+style) row -> per-partition columns via K=1 matmuls (a column
            # DMA would cost 128 descriptors ~= 15us on a jammed queue)
            sty1 = setup.tile([1, D], f32)
            nc.scalar.activation(sty1[:], styrow[:], AF.Copy, bias=1.0, scale=1.0)
            sty1b = setup.tile([1, D], bf16)
            with nc.allow_low_precision(reason="style factors; bf16 matches bank"):
                nc.vector.tensor_copy(sty1b[:], sty1[:])
            ones11_b = bt0[0:1, BROW + 129:BROW + 130]
            styps = normps.tile([128, IC], f32, tag="aux")
            stycols = []
            for ic in range(IC):
                nc.tensor.matmul(styps[:, ic:ic + 1],
                                 sty1b[0:1, ic * 128:(ic + 1) * 128],
                                 ones11_b, start=True, stop=True)
                sc = setup.tile([128, 1], f32, tag=f"sty{ic}")
                nc.scalar.activation(sc[:], styps[:, ic:ic + 1], AF.Copy,
                                     bias=0.0, scale=1.0)
                stycols.append(sc)

            # ones column for the cross-partition (i) reduction matmul
            ones_r = setup.tile([128, 1], f32r)
            nc.vector.tensor_copy(ones_r[:], onescol)
            ones12 = setup.tile([1, 2], f32)
            nc.vector.memset(ones12[:], 1.0)

            # ---------- mixing / norm / conv, flattened for PE-stream order:
            # mix(oc0) -> conv0 s0-5 -> mix(oc1,ic1) -> conv0 s6-7 -> conv1.
            # The DVE MAC chain for (oc1,ic0) is traced before mix(oc1,ic1)
            # so it starts as soon as its tiles land, fully inside conv0.
            SL = ((0, 512), (512, 1024), (1024, OCK))
            km = {}
            normcols = {}

            def mix_pe(oc, ic):
                kt = kernp.tile([128, OCK], f32r, tag=f"kern{oc}{ic}", name=f"kt{oc}{ic}")
                ps0 = mixps.tile([128, 512], f32, tag="mix0", name=f"m0{oc}{ic}")
                ps1 = mixps.tile([128, 512], f32, tag="mix1", name=f"m1{oc}{ic}")
                ps2 = mixps.tile([128, OCK - 1024], f32, tag="mix2", name=f"m2{oc}{ic}")
                pss = (ps0, ps1, ps2)
                for f in range(F):
                    bt = bts[(oc, ic, f // FPQ)]
                    fo = (f % FPQ) * OCK
                    for (lo, hi), ps in zip(SL, pss):
                        nc.tensor.matmul(ps[:], diags[f][:],
                                         bt[:, fo + lo:fo + hi],
                                         start=(f == 0), stop=(f == F - 1))
                for (lo, hi), ps in zip(SL, pss):
                    nc.vector.tensor_scalar(
                        out=kt[:, lo:hi], in0=ps[:], scalar1=stycols[ic][:],
                        scalar2=None, op0=AluOpType.mult)
                km[(ic, oc)] = kt

            def mix_mac(oc, ic):
                kt = kernp.tile([128, OCK], f32r, tag=f"kern{oc}{ic}", name=f"kt{oc}{ic}")
                acc0 = kernp.tile([128, OCK], f32, tag="macacc0", name="macacc0")
                acc1 = kernp.tile([128, OCK], f32, tag="macacc1", name="macacc1")
                accs = (acc0, acc1)
                with nc.allow_low_precision(reason="bf16 in, f32 acc"):
                    nc.vector.tensor_scalar(
                        out=accs[0][:], in0=bts[(oc, ic, 0)][:, 0:OCK],
                        scalar1=wbc[:, 0:1], scalar2=None, op0=AluOpType.mult)
                    for f in range(1, F):
                        bt = bts[(oc, ic, f // FPQ)]
                        fo = (f % FPQ) * OCK
                        nc.vector.scalar_tensor_tensor(
                            out=accs[f % 2][:], in0=bt[:, fo:fo + OCK],
                            scalar=wbc[:, f:f + 1], in1=accs[(f + 1) % 2][:],
                            op0=AluOpType.mult, op1=AluOpType.add)
                nc.vector.tensor_scalar(
                    out=kt[:], in0=accs[(F - 1) % 2][:],
                    scalar1=stycols[ic][:], scalar2=None, op0=AluOpType.mult)
                km[(ic, oc)] = kt

            def demod_dve(oc, ic):
                # square + reduce-over-khw partials (DVE only)
                kt = km[(ic, oc)]
                scr = kernp.tile([128, OCK], f32r, tag="sqscratch", name=f"scr{oc}{ic}")
                nc.vector.tensor_mul(scr[:], kt[:], kt[:])
                redk = kernp.tile([128, 128], f32r, tag=f"redk{oc}{ic}", name=f"redk{oc}{ic}")
                with nc.allow_low_precision(reason="f32r is 4-byte"):
                    nc.vector.tensor_reduce(
                        redk[:], scr[:, :].rearrange("p (o r) -> p o r", r=KHW),
                        axis=mybir.AxisListType.X, op=AluOpType.add)
                return redk

            def demod_pe(npsum, redk, first, last):
                nc.tensor.matmul(npsum[:], ones_r[:], redk[:],
                                 start=first, stop=last)

            def norm_final(oc, npsum):
                nrow = setup.tile([1, 128], f32, tag=f"nrow{oc}", name=f"nrow{oc}")
                nc.vector.tensor_scalar_add(nrow[:], npsum[:], 1e-8)
                nsq = setup.tile([1, 128], f32, tag=f"nsq{oc}", name=f"nsq{oc}")
                nc.scalar.activation(nsq[:], nrow[:], AF.Sqrt, bias=0.0, scale=1.0)
                nrec = setup.tile([1, 128], f32, tag=f"nrec{oc}", name=f"nrec{oc}")
                nc.vector.reciprocal(nrec[:], nsq[:])
                ntr = normps.tile([128, 2], f32, tag="aux", name=f"ntr{oc}")
                nc.tensor.matmul(ntr[:], nrec[:], ones12[:], start=True, stop=True)
                ncol = setup.tile([128, 1], f32, tag=f"ncol{oc}", name=f"ncol{oc}")
                nc.scalar.activation(ncol[:], ntr[:, 0:1], AF.Copy, bias=0.0, scale=1.0)
                normcols[oc] = ncol

            def conv(oc, s_lo, s_hi):
                ic_order = (0, 1) if oc == 0 else (1, 0)
                for s in range(s_lo, s_hi):
                    r0 = s * SROWS
                    cps = convps.tile([128, SN], f32, tag="conv", name=f"c{oc}{s}")
                    first = True
                    for ici, ic in enumerate(ic_order):
                        xv = xpads[ic][:, :].rearrange("p (r c) -> p r c", c=PW)
                        kv = km[(ic, oc)][:, :].rearrange("p (o r) -> p o r", r=KHW)
                        for kh in range(KK):
                            for kw in range(KK):
                                rhs = xv[:, r0 + kh: r0 + kh + SROWS, kw:kw + W]
                                nc.tensor.matmul(
                                    cps[:], kv[:, :, kh * KK + kw], rhs,
                                    start=first,
                                    stop=(ici == IC - 1 and kh == KK - 1 and kw == KK - 1))
                                first = False
                    yt = youtp.tile([128, SN], f32, tag="y", name=f"y{oc}{s}")
                    nc.scalar.activation(yt[:], cps[:], AF.Copy,
                                         bias=0.0, scale=normcols[oc][:])
                    nc.gpsimd.dma_start(
                        y_d[oc * 128:(oc + 1) * 128, r0 * W:(r0 + SROWS) * W], yt[:])

            # oc0: mix both halves, norm, first 6 conv tiles
            npsum0 = normps.tile([1, 128], f32, tag="norm", name="npsum0")
            mix_pe(0, 0)
            demod_pe(npsum0, demod_dve(0, 0), True, False)
            mix_pe(0, 1)
            demod_pe(npsum0, demod_dve(0, 1), False, True)
            norm_final(0, npsum0)
            conv(0, 0, 6)

            # oc1 mixing lives inside conv0's tail: the DVE chain first (it
            # only needs its DMAs), then the PE block, then the norm matmuls
            npsum1 = normps.tile([1, 128], f32, tag="norm", name="npsum1")
            mix_mac(1, 0)
            redk10 = demod_dve(1, 0)
            mix_pe(1, 1)
            redk11 = demod_dve(1, 1)
            demod_pe(npsum1, redk11, True, False)
            demod_pe(npsum1, redk10, False, True)
            conv(0, 6, NS)
            norm_final(1, npsum1)
            conv(1, 0, NS)

    nc.compile()
    return nc


def _get_compiled():
    global _COMPILED
    if _COMPILED is None:
        _COMPILED = _build()
    return _COMPILED


def _make_in_maps(x, bank_request, style, bank_weight):
    # bank: (F, O, I, KH, KW) -> [oc, ic, fq, i, f, o_local, khw] bf16
    bf16_np = mybir.dt.np(mybir.dt.bfloat16)
    A = bank_weight.astype(np.float32).reshape(FQ, FPQ, OC, 128, IC, 128, KHW)
    #                     dims: (fq, f, oc, o_local, ic, i, khw)
    core = A.transpose(2, 4, 0, 5, 1, 3, 6).reshape(OC * IC * FQ * 128, BROW)
    bankT = np.zeros((OC * IC * FQ * 128, BROW + 257), dtype=np.float32)
    bankT[:, 0:BROW] = core
    bankT[0:128, BROW:BROW + 128] = np.eye(128, dtype=np.float32)
    bankT[0:128, BROW + 128] = 1.0
    bankT[0, BROW + 129:BROW + 257] = 1.0
    bankT = np.ascontiguousarray(bankT).astype(bf16_np)
    maps = []
    xpad = np.zeros((B, D, PH_, PW), dtype=np.float32)
    xpad[:, :, 1:1 + H, 1:1 + W] = x.astype(np.float32).reshape(B, D, H, W)
    for b in range(B):
        maps.append({
            "x": np.ascontiguousarray(xpad[b].reshape(D, PH_ * PW)),
            "bank": bankT,
            "breq": np.ascontiguousarray(
                bank_request[b].astype(np.float32).reshape(1, F)),
            "sty": np.ascontiguousarray(style[b].astype(np.float32).reshape(1, D)),
        })
    return maps


def run(inputs, trace=False, **trace_kwargs):
    nc = _get_compiled()
    in_maps = _make_in_maps(inputs["x"], inputs["bank_request"],
                            inputs["style"], inputs["bank_weight"])
    # The first execution of a freshly compiled NEFF occasionally dies with
    # NRT_EXEC_UNIT_UNRECOVERABLE on this runtime; a plain retry succeeds.
    last_exc = None
    for _ in range(3):
        try:
            res = run_bass_kernel_spmd(nc, in_maps, core_ids=list(range(B)),
                                       trace=trace, **trace_kwargs)
            y = np.stack([res.results[b]["y"].reshape(D, H, W) for b in range(B)],
                         axis=0)
            return y, res
        except Exception as e:  # noqa: BLE001
            last_exc = e
    raise last_exc


def kernel(x, bank_request, style, bank_weight):
    y, _ = run({"x": np.asarray(x), "bank_request": np.asarray(bank_request),
                "style": np.asarray(style), "bank_weight": np.asarray(bank_weight)})
    return y



# revision 3
# speedup vs baseline: 1.0221x; 1.0221x over previous
"""BankModulatedConv Trainium2 kernel.

Problem (per sample b of B=8):
  w = softmax(bank_request[b])                        # (16,)
  kern = sum_f w[f] * bank_weight[f]                  # (o, i, kh, kw) = (256, 256, 3, 3)
  kern *= (1 + style[b, i])                           # input-channel modulation
  kern *= rsqrt(sum_{i,kh,kw} kern^2 + 1e-8)          # per-o L2 demodulation
  y[b] = conv2d(x[b], kern, stride 1, SAME)           # (256, 64, 64)

Mapping: the work grid is 8 samples x 2 o-chunks = 16 units; core c takes
o-chunk (c % 2) for the sample pair (2*(c//2), 2*(c//2)+1).  This halves the
per-core filter-bank DMA (9.4 MB bf16 instead of 18.9 MB replicated) while
keeping every conv matmul at full M=128, and demodulation stays core-local
(the per-o L2 sum runs over (i, khw) which this core holds completely).

Per core:
  - bank ships host-rearranged as [ic(2), fp(8), i(128), j(2), o_local(128),
    khw(9)] bf16 so each DMA row is 4608 contiguous bytes; the first tile
    carries 257 extra constant columns (I_128, ones) so constants arrive on
    fat rows too.  bf16 is storage precision only -- mixing accumulates f32.
  - sample s0 mixes on TensorE (16 accumulated diag(w_f) matmuls per psum
    slice); sample s1 mixes as a scalar_tensor_tensor MAC chain on VectorE,
    paced tile-by-tile with the PE stream so both finish with the bank DMA.
  - style modulation (1+style[i]) is fused into the PSUM/acc -> bf16 kernel
    copy on ScalarE; softmax weights / style / demod scales are spread
    across partitions with tiny K=1 matmuls (never column DMAs).
  - demod: square + reduce-over-khw on DVE, then a ones-vector matmul
    reduces across the i partition dim; the rsqrt'd scale is applied per
    output channel in the ScalarE conv-PSUM-out copy.
  - conv runs in bf16 (x is host-padded and bf16-cast; kernel tiles are
    bf16): per spatial group of <=3 row-tiles, 18 accumulated matmuls
    (i_chunk x 3 x 3) share each lhsT load.  x arrives as overlapping
    10-row strips so the first conv tile only waits for its own rows.
"""
import sys

if "/opt/trn_rl_repo" not in sys.path:
    sys.path.insert(0, "/opt/trn_rl_repo")

import numpy as np
import concourse.bacc as bacc
import concourse.mybir as mybir
import concourse.tile as tile
from concourse.alu_op_type import AluOpType
from concourse.bass_utils import run_bass_kernel_spmd

dt = mybir.dt
AF = mybir.ActivationFunctionType

B, F, D, KK, H, W = 8, 16, 256, 3, 64, 64
HW = H * W            # 4096
KHW = KK * KK         # 9
IC = D // 128         # 2 i-chunks
FP = F // 2           # 8 f-pair bank tiles per i-chunk
OCK = 128 * KHW       # 1152 free elems per khw group
BROW = 2 * OCK        # 2304 bf16 elems per bank DMA row (f-pair)
PW = W + 2            # padded width 66
PH_ = H + 2           # padded height 66
NS = 8                # spatial tiles (8 rows each)
SROWS = H // NS       # 8 rows per spatial tile
SN = SROWS * W        # 512 = conv matmul moving size
SCOLS = (SROWS + 2) * PW  # 660 cols per x strip (10 padded rows)
CTL = 2 * F + 2 * D   # control row: breq s0|s1, style s0|s1

_COMPILED = None


def _build(num_devices=B):
    nc = bacc.Bacc("TRN2", target_bir_lowering=False, debug=False,
                   num_devices=num_devices)

    # x: both samples, host-padded + bf16, rows = (s, ic) blocks of 128
    x_d = nc.dram_tensor("x", [2 * D, PH_ * PW], dt.bfloat16,
                         kind="ExternalInput").ap()
    # bank: this core's o-chunk; first 128 rows carry BROW+257 columns whose
    # trailing 257 are constants (I_128, ones column, ones row on partition 0)
    bank_d = nc.dram_tensor("bank", [IC * FP * 128, BROW + 257], dt.bfloat16,
                            kind="ExternalInput").ap()
    ctl_d = nc.dram_tensor("ctl", [1, CTL], dt.float32, kind="ExternalInput").ap()
    y_d = nc.dram_tensor("y", [2 * 128, HW], dt.float32, kind="ExternalOutput").ap()

    f32, f32r, bf16 = dt.float32, dt.float32r, dt.bfloat16

    with tile.TileContext(nc) as tc:
        with (
            tc.tile_pool(name="setup", bufs=1) as setup,
            tc.tile_pool(name="xp", bufs=1) as xp,
            tc.tile_pool(name="bankp", bufs=4) as bankp,
            tc.tile_pool(name="kern", bufs=1) as kernp,
            tc.tile_pool(name="yout", bufs=4) as youtp,
            tc.tile_pool(name="mixps", bufs=1, space="PSUM") as mixps,
            tc.tile_pool(name="convps", bufs=3, space="PSUM") as convps,
            tc.tile_pool(name="auxps", bufs=1, space="PSUM") as auxps,
        ):
            # tiny control DMA first so it lands before the bank megabytes
            ctl = setup.tile([1, CTL], f32)
            nc.sync.dma_start(ctl[:], ctl_d[:])

            # ---------- bank DMAs, in consumption order ----------
            bts = {}

            def issue_bank_dma(ic, fp):
                wide = ic == 0 and fp == 0
                tag = "bankw" if wide else "bank"
                cols = BROW + 257 if wide else BROW
                bt = bankp.tile([128, cols], bf16, tag=tag)
                row0 = (ic * FP + fp) * 128
                nc.sync.dma_start(bt[:], bank_d[row0:row0 + 128, 0:cols])
                bts[(ic, fp)] = bt

            for ic in range(IC):
                for fp in range(FP):
                    issue_bank_dma(ic, fp)

            # ---------- x strips: 10 padded rows per spatial tile ----------
            # issued after the bank so the mix->conv critical path owns the
            # early bandwidth; conv tile t only needs strip t of each chunk.
            xstr = {}
            for s in range(2):
                for t in range(NS):
                    for ic in range(IC):
                        st = xp.tile([128, SCOLS], bf16, tag=f"x{s}{ic}{t}")
                        row0 = (s * IC + ic) * 128
                        c0 = t * SROWS * PW
                        nc.sync.dma_start(
                            st[:], x_d[row0:row0 + 128, c0:c0 + SCOLS])
                        xstr[(s, ic, t)] = st

            # ---------- setup: softmax weights, diag tiles, style columns ----
            bt0 = bts[(0, 0)]
            ident = bt0[:, BROW:BROW + 128]            # I_128 (bf16-exact)
            onescol = bt0[:, BROW + 128:BROW + 129]
            onesrow_b = bt0[0:1, BROW + 129:BROW + 257]
            ones11_b = bt0[0:1, BROW + 129:BROW + 130]

            # softmax for both samples, all on partition 0 (no max-shift:
            # inputs are O(1) and f32 exp only overflows past ~88)
            ex = setup.tile([1, 2 * F], f32)
            nc.scalar.activation(ex[:], ctl[:, 0:2 * F], AF.Exp, bias=0.0, scale=1.0)
            wrow = setup.tile([1, 2 * F], f32)
            for s in range(2):
                sm = setup.tile([1, 1], f32, tag=f"sm{s}")
                nc.vector.reduce_sum(sm[:], ex[:, s * F:(s + 1) * F],
                                     axis=mybir.AxisListType.X)
                rs = setup.tile([1, 1], f32, tag=f"rs{s}")
                nc.vector.reciprocal(rs[:], sm[:])
                nc.vector.tensor_scalar(out=wrow[:, s * F:(s + 1) * F],
                                        in0=ex[:, s * F:(s + 1) * F],
                                        scalar1=rs[:], scalar2=None,
                                        op0=AluOpType.mult)
            wrow_b = setup.tile([1, 2 * F], bf16)
            with nc.allow_low_precision(reason="broadcast only"):
                nc.vector.tensor_copy(wrow_b[:], wrow[:])
            # broadcast w across partitions with a K=1 bf16 matmul
            wbps = auxps.tile([128, 2 * F], f32, tag="aux")
            nc.tensor.matmul(wbps[:], onesrow_b[:], wrow_b[:], start=True, stop=True)
            wbc = setup.tile([128, 2 * F], f32)
            nc.vector.tensor_copy(wbc[:], wbps[:])

            # per-f diagonal lhsT tiles diag(w_f) for the s0 PE mix
            diags = []
            with nc.allow_low_precision(reason="bf16 diag weights; mix accumulates f32"):
                for f in range(F):
                    dg = setup.tile([128, 128], bf16, tag=f"diag{f}")
                    nc.vector.tensor_scalar(out=dg[:], in0=ident[:],
                                            scalar1=wbc[:, f:f + 1],
                                            scalar2=None, op0=AluOpType.mult)
                    diags.append(dg)

            # (1 + style[i]) as per-partition columns via K=1 matmuls
            sty1 = setup.tile([1, 2 * D], f32)
            nc.scalar.activation(sty1[:], ctl[:, 2 * F:CTL], AF.Copy,
                                 bias=1.0, scale=1.0)
            sty1b = setup.tile([1, 2 * D], bf16)
            with nc.allow_low_precision(reason="style factors; bf16 matches bank"):
                nc.vector.tensor_copy(sty1b[:], sty1[:])
            stycols = {}
            styps = auxps.tile([128, 4], f32, tag="aux", name="styps")
            for s in range(2):
                for ic in range(IC):
                    k = s * IC + ic
                    nc.tensor.matmul(styps[:, k:k + 1],
                                     sty1b[0:1, s * D + ic * 128:s * D + (ic + 1) * 128],
                                     ones11_b, start=True, stop=True)
                    sc = setup.tile([128, 1], f32, tag=f"sty{s}{ic}")
                    nc.scalar.activation(sc[:], styps[:, k:k + 1], AF.Copy,
                                         bias=0.0, scale=1.0)
                    stycols[(s, ic)] = sc

            # ones column for the cross-partition (i) reduction matmul
            ones_r = setup.tile([128, 1], f32r)
            nc.vector.tensor_copy(ones_r[:], onescol)
            ones12 = setup.tile([1, 2], f32)
            nc.vector.memset(ones12[:], 1.0)

            # ---------- mixing (s0 on PE, s1 on DVE), demod, norm ----------
            SL = ((0, 512), (512, 1024), (1024, OCK))
            km = {}
            normcols = {}

            def mix_ic(ic):
                # s0: 16 accumulated diag matmuls per psum slice
                ps = [mixps.tile([128, hi - lo], f32, tag=f"mix{k}", name=f"m{k}i{ic}")
                      for k, (lo, hi) in enumerate(SL)]
                # s1: DVE MAC chain, interleaved per f-pair tile with the PE
                # stream so both consume tiles at arrival rate
                acc0 = kernp.tile([128, OCK], f32, tag="macacc0", name=f"acc0i{ic}")
                acc1 = kernp.tile([128, OCK], f32, tag="macacc1", name=f"acc1i{ic}")
                accs = (acc0, acc1)
                for fp in range(FP):
                    bt = bts[(ic, fp)]
                    for j in range(2):
                        f = 2 * fp + j
                        fo = j * OCK
                        for (lo, hi), p in zip(SL, ps):
                            nc.tensor.matmul(p[:], diags[f][:],
                                             bt[:, fo + lo:fo + hi],
                                             start=(f == 0), stop=(f == F - 1))
                        with nc.allow_low_precision(reason="bf16 in, f32 acc"):
                            if f == 0:
                                nc.vector.tensor_scalar(
                                    out=accs[0][:], in0=bt[:, fo:fo + OCK],
                                    scalar1=wbc[:, F:F + 1], scalar2=None,
                                    op0=AluOpType.mult)
                            else:
                                nc.vector.scalar_tensor_tensor(
                                    out=accs[f % 2][:], in0=bt[:, fo:fo + OCK],
                                    scalar=wbc[:, F + f:F + f + 1],
                                    in1=accs[(f + 1) % 2][:],
                                    op0=AluOpType.mult, op1=AluOpType.add)
                # style fused into the kernel copy (ScalarE, bf16 out)
                kt0 = kernp.tile([128, OCK], bf16, tag=f"kern0{ic}", name=f"kt0{ic}")
                with nc.allow_low_precision(reason="conv runs bf16"):
                    for (lo, hi), p in zip(SL, ps):
                        nc.scalar.activation(kt0[:, lo:hi], p[:], AF.Copy,
                                             bias=0.0, scale=stycols[(0, ic)][:])
                    km[(0, ic)] = kt0
                    kt1 = kernp.tile([128, OCK], bf16, tag=f"kern1{ic}", name=f"kt1{ic}")
                    nc.scalar.activation(kt1[:], accs[(F - 1) % 2][:], AF.Copy,
                                         bias=0.0, scale=stycols[(1, ic)][:])
                    km[(1, ic)] = kt1

            def demod_dve(s, ic):
                # square + reduce-over-khw partials (DVE only)
                kt = km[(s, ic)]
                scr = kernp.tile([128, OCK], f32r, tag="sqscratch", name=f"scr{s}{ic}")
                with nc.allow_low_precision(reason="bf16 kernel squared into f32"):
                    nc.vector.tensor_mul(scr[:], kt[:], kt[:])
                redk = kernp.tile([128, 128], f32r, tag=f"redk{s}{ic}",
                                  name=f"redk{s}{ic}")
                with nc.allow_low_precision(reason="f32r is 4-byte"):
                    nc.vector.tensor_reduce(
                        redk[:], scr[:, :].rearrange("p (o r) -> p o r", r=KHW),
                        axis=mybir.AxisListType.X, op=AluOpType.add)
                return redk

            def norm_final(s, npsum):
                nrow = setup.tile([1, 128], f32, tag=f"nrow{s}", name=f"nrow{s}")
                nc.vector.tensor_scalar_add(nrow[:], npsum[:], 1e-8)
                nsq = setup.tile([1, 128], f32, tag=f"nsq{s}", name=f"nsq{s}")
                nc.scalar.activation(nsq[:], nrow[:], AF.Sqrt, bias=0.0, scale=1.0)
                nrec = setup.tile([1, 128], f32, tag=f"nrec{s}", name=f"nrec{s}")
                nc.vector.reciprocal(nrec[:], nsq[:])
                ntr = auxps.tile([128, 2], f32, tag="aux", name=f"ntr{s}")
                nc.tensor.matmul(ntr[:], nrec[:], ones12[:], start=True, stop=True)
                ncol = setup.tile([128, 1], f32, tag=f"ncol{s}", name=f"ncol{s}")
                nc.scalar.activation(ncol[:], ntr[:, 0:1], AF.Copy, bias=0.0, scale=1.0)
                normcols[s] = ncol

            def conv(s, groups):
                for group in groups:
                    cps = [convps.tile([128, SN], f32, tag="conv", name=f"c{s}{t}")
                           for t in group]
                    first = True
                    for ic in range(IC):
                        kv = km[(s, ic)][:, :].rearrange("p (o r) -> p o r", r=KHW)
                        for kh in range(KK):
                            for kw in range(KK):
                                last = (ic == IC - 1 and kh == KK - 1 and kw == KK - 1)
                                for t, cp in zip(group, cps):
                                    xv = xstr[(s, ic, t)][:, :].rearrange(
                                        "p (r c) -> p r c", c=PW)
                                    nc.tensor.matmul(
                                        cp[:], kv[:, :, kh * KK + kw],
                                        xv[:, kh:kh + SROWS, kw:kw + W],
                                        start=first, stop=last)
                                first = False
                    for t, cp in zip(group, cps):
                        yt = youtp.tile([128, SN], f32, tag="y", name=f"y{s}{t}")
                        nc.scalar.activation(yt[:], cp[:], AF.Copy,
                                             bias=0.0, scale=normcols[s][:])
                        nc.gpsimd.dma_start(
                            y_d[s * 128:(s + 1) * 128, t * SN:(t + 1) * SN], yt[:])

            npsum0 = auxps.tile([1, 128], f32, tag="aux", name="npsum0")
            mix_ic(0)
            demod_pe0 = demod_dve(0, 0)
            nc.tensor.matmul(npsum0[:], ones_r[:], demod_pe0[:], start=True, stop=False)
            mix_ic(1)
            nc.tensor.matmul(npsum0[:], ones_r[:], demod_dve(0, 1)[:],
                             start=False, stop=True)
            norm_final(0, npsum0)
            conv(0, [(0, 1, 2), (3, 4, 5)])
            npsum1 = auxps.tile([1, 128], f32, tag="aux", name="npsum1")
            nc.tensor.matmul(npsum1[:], ones_r[:], demod_dve(1, 0)[:],
                             start=True, stop=False)
            nc.tensor.matmul(npsum1[:], ones_r[:], demod_dve(1, 1)[:],
                             start=False, stop=True)
            norm_final(1, npsum1)
            conv(0, [(6, 7)])
            conv(1, [(0, 1, 2), (3, 4, 5), (6, 7)])

    nc.compile()
    return nc


def _get_compiled():
    global _COMPILED
    if _COMPILED is None:
        _COMPILED = _build()
    return _COMPILED


def _make_in_maps(x, bank_request, style, bank_weight):
    bf16_np = mybir.dt.np(mybir.dt.bfloat16)
    # bank: (F, O, I, KH, KW) -> per-oc [ic, fp, i, j, o_local, khw] bf16
    A = bank_weight.astype(np.float32).reshape(FP, 2, 2, 128, IC, 128, KHW)
    #                     dims: (fp, j, oc, o_local, ic, i, khw)
    banks = []
    for oc in range(2):
        core = A[:, :, oc].transpose(3, 0, 4, 1, 2, 5).reshape(IC * FP * 128, BROW)
        bankT = np.zeros((IC * FP * 128, BROW + 257), dtype=np.float32)
        bankT[:, 0:BROW] = core
        bankT[0:128, BROW:BROW + 128] = np.eye(128, dtype=np.float32)
        bankT[0:128, BROW + 128] = 1.0
        bankT[0, BROW + 129:BROW + 257] = 1.0
        banks.append(np.ascontiguousarray(bankT).astype(bf16_np))

    xpad = np.zeros((B, D, PH_, PW), dtype=np.float32)
    xpad[:, :, 1:1 + H, 1:1 + W] = x.astype(np.float32).reshape(B, D, H, W)
    xpad = xpad.reshape(B, D, PH_ * PW).astype(bf16_np)

    breq = bank_request.astype(np.float32)
    sty = style.astype(np.float32).reshape(B, D)

    maps = []
    for c in range(B):
        oc = c % 2
        s0 = 2 * (c // 2)
        ctl = np.concatenate([breq[s0], breq[s0 + 1], sty[s0], sty[s0 + 1]])
        maps.append({
            "x": np.ascontiguousarray(xpad[s0:s0 + 2].reshape(2 * D, PH_ * PW)),
            "bank": banks[oc],
            "ctl": np.ascontiguousarray(ctl.reshape(1, CTL)),
        })
    return maps


def run(inputs, trace=False, **trace_kwargs):
    nc = _get_compiled()
    in_maps = _make_in_maps(inputs["x"], inputs["bank_request"],
                            inputs["style"], inputs["bank_weight"])
    # The first execution of a freshly compiled NEFF occasionally dies with
    # NRT_EXEC_UNIT_UNRECOVERABLE on this runtime; a plain retry succeeds.
    last_exc = None
    for _ in range(3):
        try:
            res = run_bass_kernel_spmd(nc, in_maps, core_ids=list(range(B)),
                                       trace=trace, **trace_kwargs)
            y = np.empty((B, D, H, W), dtype=np.float32)
            for c in range(B):
                oc = c % 2
                s0 = 2 * (c // 2)
                yc = res.results[c]["y"].reshape(2, 128, H, W)
                y[s0, oc * 128:(oc + 1) * 128] = yc[0]
                y[s0 + 1, oc * 128:(oc + 1) * 128] = yc[1]
            return y, res
        except Exception as e:  # noqa: BLE001
            last_exc = e
    raise last_exc


def kernel(x, bank_request, style, bank_weight):
    y, _ = run({"x": np.asarray(x), "bank_request": np.asarray(bank_request),
                "style": np.asarray(style), "bank_weight": np.asarray(bank_weight)})
    return y


# revision 13
# speedup vs baseline: 1.0376x; 1.0151x over previous
"""BankModulatedConv Trainium2 kernel.

Problem (per sample b of B=8):
  w = softmax(bank_request[b])                        # (16,)
  kern = sum_f w[f] * bank_weight[f]                  # (o, i, kh, kw) = (256, 256, 3, 3)
  kern *= (1 + style[b, i])                           # input-channel modulation
  kern *= rsqrt(sum_{i,kh,kw} kern^2 + 1e-8)          # per-o L2 demodulation
  y[b] = conv2d(x[b], kern, stride 1, SAME)           # (256, 64, 64)

Mapping: the work grid is 8 samples x 2 o-chunks = 16 units; core c takes
o-chunk (c % 2) for the sample pair (2*(c//2), 2*(c//2)+1).  This halves the
per-core filter-bank DMA (9.4 MB bf16 instead of 18.9 MB replicated) while
keeping every conv matmul at full M=128, and demodulation stays core-local
(the per-o L2 sum runs over (i, khw) which this core holds completely).

Per core:
  - bank ships host-rearranged as [ic(2), fp(8), i(128), j(2), khw(9),
    o_local(128)] bf16: each DMA row is 4608 contiguous bytes, and the
    khw-major column order means the conv lhsT slice for one (kh, kw) tap
    is a fully contiguous [128, 128] block (a strided lhsT fetch costs
    ~35 ns per matmul in exposed ldweights time).  Constants (I_128, ones)
    ride in a separate tiny [128, 257] DMA issued first.
  - mixing is split per f across engines to shorten the serial window:
    PE takes cols 0:768 of each 1152-col khw/o block (16 accumulated
    diag(w_f) matmuls into two PSUM slices per sample), DVE takes cols
    768:1152 as a scalar_tensor_tensor MAC chain.  Both are paced by the
    bank tile arrivals; bf16 is storage precision only (f32 accumulate).
  - style modulation (1+style[i]) is fused into the PSUM/acc -> bf16
    kernel copies on ScalarE; softmax weights / style / demod scales are
    spread across partitions with tiny K=1 matmuls (never column DMAs).
  - demod: square + reduce-over-khw (strided view) on DVE, then a
    ones-vector matmul reduces across the i partition dim; the rsqrt'd
    scale is applied per output channel in the ScalarE conv-PSUM-out copy.
    The tiny demod matmuls are emitted after the first conv groups so the
    PE never stalls waiting on DVE latency.
  - conv runs in bf16: per spatial group of <=3 row-tiles, 18 accumulated
    matmuls (i_chunk x 3 x 3) with contiguous lhsT.  x is host-padded,
    bf16-cast, and arrives as overlapping 10-row strips interleaved into
    the second half of the bank stream.
"""
import sys

if "/opt/trn_rl_repo" not in sys.path:
    sys.path.insert(0, "/opt/trn_rl_repo")

import numpy as np
import concourse.bacc as bacc
import concourse.mybir as mybir
import concourse.tile as tile
from concourse.alu_op_type import AluOpType
from concourse.bass_utils import run_bass_kernel_spmd

dt = mybir.dt
AF = mybir.ActivationFunctionType

B, F, D, KK, H, W = 8, 16, 256, 3, 64, 64
HW = H * W            # 4096
KHW = KK * KK         # 9
IC = D // 128         # 2 i-chunks
FP = F // 2           # 8 f-pair bank tiles per i-chunk
OCK = 128 * KHW       # 1152 free elems per (j) block, khw-major: col = khw*128+o
BROW = 2 * OCK        # 2304 bf16 elems per bank DMA row (f-pair)
PESPL = ((0, 512), (512, 768))   # PE mix slices per f-block
DVLO, DVHI = 768, OCK            # DVE mix slice per f-block
PW = W + 2            # padded width 66
PH_ = H + 2           # padded height 66
NS = 8                # spatial tiles (8 rows each)
SROWS = H // NS       # 8 rows per spatial tile
SN = SROWS * W        # 512 = conv matmul moving size
SCOLS = (SROWS + 2) * PW  # 660 cols per x strip (10 padded rows)
CTL = 2 * F + 2 * D   # control row: breq s0|s1, style s0|s1
CGROUPS = ((0, 1, 2), (3, 4, 5), (6, 7))

_COMPILED = None


def _build(num_devices=B):
    nc = bacc.Bacc("TRN2", target_bir_lowering=False, debug=False,
                   num_devices=num_devices)

    # x: both samples, host-padded + bf16, rows = (s, ic) blocks of 128
    x_d = nc.dram_tensor("x", [2 * D, PH_ * PW], dt.bfloat16,
                         kind="ExternalInput").ap()
    # bank: this core's o-chunk; cols 0:BROW are the mix payload, cols
    # BROW:BROW+257 (first 128 rows) carry I_128 / ones constants
    bank_d = nc.dram_tensor("bank", [IC * FP * 128, BROW + 257], dt.bfloat16,
                            kind="ExternalInput").ap()
    ctl_d = nc.dram_tensor("ctl", [1, CTL], dt.float32, kind="ExternalInput").ap()
    y_d = nc.dram_tensor("y", [2 * 128, HW], dt.float32, kind="ExternalOutput").ap()

    f32, f32r, bf16 = dt.float32, dt.float32r, dt.bfloat16

    with tile.TileContext(nc) as tc:
        with (
            tc.tile_pool(name="setup", bufs=1) as setup,
            tc.tile_pool(name="xp", bufs=1) as xp,
            tc.tile_pool(name="bankp", bufs=4) as bankp,
            tc.tile_pool(name="kern", bufs=1) as kernp,
            tc.tile_pool(name="yout", bufs=4) as youtp,
            tc.tile_pool(name="mixps", bufs=1, space="PSUM") as mixps,
            tc.tile_pool(name="convps", bufs=3, space="PSUM") as convps,
            tc.tile_pool(name="auxps", bufs=1, space="PSUM") as auxps,
        ):
            # tiny control + constants DMAs first, ahead of the bank megabytes
            ctl = setup.tile([1, CTL], f32)
            nc.sync.dma_start(ctl[:], ctl_d[:])
            consts = setup.tile([128, 257], bf16, tag="consts")
            nc.sync.dma_start(consts[:], bank_d[0:128, BROW:BROW + 257])

            bts = {}

            def issue_bank_dma(ic, fp):
                bt = bankp.tile([128, BROW], bf16, tag="bank")
                row0 = (ic * FP + fp) * 128
                nc.sync.dma_start(bt[:], bank_d[row0:row0 + 128, 0:BROW])
                bts[(ic, fp)] = bt

            xstr = {}

            def issue_x_strip(s, ic, t):
                st = xp.tile([128, SCOLS], bf16, tag=f"x{s}{ic}{t}")
                row0 = (s * IC + ic) * 128
                c0 = t * SROWS * PW
                nc.sync.dma_start(st[:], x_d[row0:row0 + 128, c0:c0 + SCOLS])
                xstr[(s, ic, t)] = st

            # bank ic0, then bank ic1 with s0's x strips woven in, then x s1:
            # conv(s0) starts right after mixing, so its x must be resident by
            # the end of the bank stream; s1's x can trail under conv(s0).
            for fp in range(FP):
                issue_bank_dma(0, fp)
            for fp in range(FP):
                issue_bank_dma(1, fp)
                issue_x_strip(0, 0, fp)
                issue_x_strip(0, 1, fp)
            for t in range(NS):
                issue_x_strip(1, 0, t)
                issue_x_strip(1, 1, t)

            # ---------- setup: softmax weights, diag tiles, style columns ----
            ident = consts[:, 0:128]                   # I_128 (bf16-exact)
            onescol = consts[:, 128:129]
            onesrow_b = consts[0:1, 129:257]
            ones11_b = consts[0:1, 129:130]

            # softmax for both samples, all on partition 0 (no max-shift:
            # inputs are O(1) and f32 exp only overflows past ~88)
            ex = setup.tile([1, 2 * F], f32)
            nc.scalar.activation(ex[:], ctl[:, 0:2 * F], AF.Exp, bias=0.0, scale=1.0)
            wrow = setup.tile([1, 2 * F], f32)
            for s in range(2):
                sm = setup.tile([1, 1], f32, tag=f"sm{s}")
                nc.vector.reduce_sum(sm[:], ex[:, s * F:(s + 1) * F],
                                     axis=mybir.AxisListType.X)
                rs = setup.tile([1, 1], f32, tag=f"rs{s}")
                nc.vector.reciprocal(rs[:], sm[:])
                nc.vector.tensor_scalar(out=wrow[:, s * F:(s + 1) * F],
                                        in0=ex[:, s * F:(s + 1) * F],
                                        scalar1=rs[:], scalar2=None,
                                        op0=AluOpType.mult)
            wrow_b = setup.tile([1, 2 * F], bf16)
            with nc.allow_low_precision(reason="broadcast only"):
                nc.vector.tensor_copy(wrow_b[:], wrow[:])
            # broadcast w across partitions with a K=1 bf16 matmul; the psum
            # lands in a mix slot (consumed before the first mix matmul)
            wbps = mixps.tile([128, 2 * F], f32, tag="mix00", name="wbps")
            nc.tensor.matmul(wbps[:], onesrow_b[:], wrow_b[:], start=True, stop=True)
            wbc = setup.tile([128, 2 * F], f32)
            nc.vector.tensor_copy(wbc[:], wbps[:])

            # per-f diagonal lhsT tiles diag(w_f) for the PE mix, per sample
            diags = {}
            with nc.allow_low_precision(reason="bf16 diag weights; mix accumulates f32"):
                for s in range(2):
                    for f in range(F):
                        dg = setup.tile([128, 128], bf16, tag=f"diag{s}{f}")
                        nc.vector.tensor_scalar(out=dg[:], in0=ident[:],
                                                scalar1=wbc[:, s * F + f:s * F + f + 1],
                                                scalar2=None, op0=AluOpType.mult)
                        diags[(s, f)] = dg

            # (1 + style[i]) as per-partition columns via K=1 matmuls
            sty1 = setup.tile([1, 2 * D], f32)
            nc.scalar.activation(sty1[:], ctl[:, 2 * F:CTL], AF.Copy,
                                 bias=1.0, scale=1.0)
            sty1b = setup.tile([1, 2 * D], bf16)
            with nc.allow_low_precision(reason="style factors; bf16 matches bank"):
                nc.vector.tensor_copy(sty1b[:], sty1[:])
            stycols = {}
            styps = mixps.tile([128, 4], f32, tag="mix01", name="styps")
            for s in range(2):
                for ic in range(IC):
                    k = s * IC + ic
                    nc.tensor.matmul(styps[:, k:k + 1],
                                     sty1b[0:1, s * D + ic * 128:s * D + (ic + 1) * 128],
                                     ones11_b, start=True, stop=True)
                    sc = setup.tile([128, 1], f32, tag=f"sty{s}{ic}")
                    nc.scalar.activation(sc[:], styps[:, k:k + 1], AF.Copy,
                                         bias=0.0, scale=1.0)
                    stycols[(s, ic)] = sc

            # ones column for the cross-partition (i) reduction matmul
            ones_r = setup.tile([128, 1], f32r)
            nc.vector.tensor_copy(ones_r[:], onescol)
            ones12 = setup.tile([1, 2], f32)
            nc.vector.memset(ones12[:], 1.0)

            # ---------- mixing / demod / norm / conv ----------
            km = {}
            redks = {}
            normcols = {}

            def mix_ic(ic):
                # PE: two PSUM slices per sample; DVE: MAC chain on the tail.
                # Every tile is padded to a full 2KB PSUM bank so no two
                # concurrent accumulation groups ever share a bank (hardware
                # start_tensor_calc state is not sub-bank safe).
                ps = {s: [mixps.tile([128, 512], f32, tag=f"mix{s}{k}",
                                     name=f"m{s}{k}i{ic}")[:, 0:hi - lo]
                          for k, (lo, hi) in enumerate(PESPL)]
                      for s in range(2)}
                accs = {s: (kernp.tile([128, DVHI - DVLO], f32, tag=f"acc{s}0",
                                       name=f"acc{s}0i{ic}"),
                            kernp.tile([128, DVHI - DVLO], f32, tag=f"acc{s}1",
                                       name=f"acc{s}1i{ic}"))
                       for s in range(2)}
                for fp in range(FP):
                    bt = bts[(ic, fp)]
                    for j in range(2):
                        f = 2 * fp + j
                        fo = j * OCK
                        for s in range(2):
                            for (lo, hi), p in zip(PESPL, ps[s]):
                                nc.tensor.matmul(p[:], diags[(s, f)][:],
                                                 bt[:, fo + lo:fo + hi],
                                                 start=(f == 0), stop=(f == F - 1))
                        with nc.allow_low_precision(reason="bf16 in, f32 acc"):
                            for s in range(2):
                                a = accs[s]
                                if f == 0:
                                    nc.vector.tensor_scalar(
                                        out=a[0][:], in0=bt[:, fo + DVLO:fo + DVHI],
                                        scalar1=wbc[:, s * F:s * F + 1],
                                        scalar2=None, op0=AluOpType.mult)
                                else:
                                    nc.vector.scalar_tensor_tensor(
                                        out=a[f % 2][:],
                                        in0=bt[:, fo + DVLO:fo + DVHI],
                                        scalar=wbc[:, s * F + f:s * F + f + 1],
                                        in1=a[(f + 1) % 2][:],
                                        op0=AluOpType.mult, op1=AluOpType.add)
                # style fused into the kernel copies (ScalarE, bf16 out)
                with nc.allow_low_precision(reason="conv runs bf16"):
                    for s in range(2):
                        kt = kernp.tile([128, OCK], bf16, tag=f"kern{s}{ic}",
                                        name=f"kt{s}{ic}")
                        sc = stycols[(s, ic)]
                        for (lo, hi), p in zip(PESPL, ps[s]):
                            nc.scalar.activation(kt[:, lo:hi], p[:], AF.Copy,
                                                 bias=0.0, scale=sc[:])
                        nc.scalar.activation(kt[:, DVLO:DVHI],
                                             accs[s][(F - 1) % 2][:], AF.Copy,
                                             bias=0.0, scale=sc[:])
                        km[(s, ic)] = kt
                # square + reduce-over-khw partials (DVE only); both samples
                # share one [128, 256] tile so a single ones-matmul reduces
                # them across the i partition dim together
                redk = kernp.tile([128, 256], f32r, tag=f"redk{ic}",
                                  name=f"redk{ic}")
                for s in range(2):
                    kt = km[(s, ic)]
                    scr = kernp.tile([128, OCK], f32r, tag="sqscratch",
                                     name=f"scr{s}{ic}")
                    with nc.allow_low_precision(reason="bf16 kernel squared into f32"):
                        nc.vector.tensor_mul(scr[:], kt[:], kt[:])
                    with nc.allow_low_precision(reason="f32r is 4-byte"):
                        nc.vector.tensor_reduce(
                            redk[:, s * 128:(s + 1) * 128],
                            scr[:, :].rearrange("p (r o) -> p o r", r=KHW),
                            axis=mybir.AxisListType.X, op=AluOpType.add)
                redks[ic] = redk

            def norm_final(s, npsum):
                nrow = setup.tile([1, 128], f32, tag=f"nrow{s}", name=f"nrow{s}")
                nc.vector.tensor_scalar_add(nrow[:], npsum[:, s * 128:(s + 1) * 128],
                                            1e-8)
                nsq = setup.tile([1, 128], f32, tag=f"nsq{s}", name=f"nsq{s}")
                nc.scalar.activation(nsq[:], nrow[:], AF.Sqrt, bias=0.0, scale=1.0)
                nrec = setup.tile([1, 128], f32, tag=f"nrec{s}", name=f"nrec{s}")
                nc.vector.reciprocal(nrec[:], nsq[:])
                # the transpose psum rides in a (long-consumed) mix slot
                ntr = mixps.tile([128, 2], f32, tag=f"mix{s}0", name=f"ntr{s}")
                nc.tensor.matmul(ntr[:], nrec[:], ones12[:], start=True, stop=True)
                ncol = setup.tile([128, 1], f32, tag=f"ncol{s}", name=f"ncol{s}")
                nc.scalar.activation(ncol[:], ntr[:, 0:1], AF.Copy, bias=0.0, scale=1.0)
                normcols[s] = ncol

            def conv_mms(s, group):
                cps = [convps.tile([128, SN], f32, tag="conv", name=f"c{s}{t}")
                       for t in group]
                for ic in range(IC):
                    kt = km[(s, ic)]
                    for k in range(KHW):
                        kh, kw = divmod(k, KK)
                        first = (ic == 0 and k == 0)
                        last = (ic == IC - 1 and k == KHW - 1)
                        for t, cp in zip(group, cps):
                            xv = xstr[(s, ic, t)][:, :].rearrange(
                                "p (r c) -> p r c", c=PW)
                            nc.tensor.matmul(
                                cp[:], kt[:, k * 128:(k + 1) * 128],
                                xv[:, kh:kh + SROWS, kw:kw + W],
                                start=first, stop=last)
                return cps

            def conv_out(s, group, cps):
                for t, cp in zip(group, cps):
                    yt = youtp.tile([128, SN], f32, tag="y", name=f"y{s}{t}")
                    nc.scalar.activation(yt[:], cp[:], AF.Copy,
                                         bias=0.0, scale=normcols[s][:])
                    nc.gpsimd.dma_start(
                        y_d[s * 128:(s + 1) * 128, t * SN:(t + 1) * SN], yt[:])

            def conv_group(s, group):
                conv_out(s, group, conv_mms(s, group))

            npsum = auxps.tile([1, 256], f32, tag="np", name="npsum")
            mix_ic(0)
            # ic0 demod reduction: PE waits briefly on DVE squares, but the
            # bank stream is the pacer here so the stall is free
            nc.tensor.matmul(npsum[:], ones_r[:], redks[0][:],
                             start=True, stop=False)
            mix_ic(1)
            # conv(s0) first; the remaining tiny demod matmuls hide behind it
            cps0 = conv_mms(0, CGROUPS[0])
            nc.tensor.matmul(npsum[:], ones_r[:], redks[1][:],
                             start=False, stop=True)
            norm_final(0, npsum)
            conv_out(0, CGROUPS[0], cps0)
            conv_group(0, CGROUPS[1])
            norm_final(1, npsum)
            conv_group(0, CGROUPS[2])
            for g in CGROUPS:
                conv_group(1, g)

    nc.compile()
    return nc


def _get_compiled():
    global _COMPILED
    if _COMPILED is None:
        _COMPILED = _build()
    return _COMPILED


def _make_in_maps(x, bank_request, style, bank_weight):
    bf16_np = mybir.dt.np(mybir.dt.bfloat16)
    # bank: (F, O, I, KH, KW) -> per-oc [ic, fp, i, j, khw, o_local] bf16
    A = bank_weight.astype(np.float32).reshape(FP, 2, 2, 128, IC, 128, KHW)
    #                     dims: (fp, j, oc, o_local, ic, i, khw)
    banks = []
    for oc in range(2):
        core = A[:, :, oc].transpose(3, 0, 4, 1, 5, 2).reshape(IC * FP * 128, BROW)
        #      (fp, j, o, ic, i, khw) -> (ic, fp, i, j, khw, o)
        bankT = np.zeros((IC * FP * 128, BROW + 257), dtype=np.float32)
        bankT[:, 0:BROW] = core
        bankT[0:128, BROW:BROW + 128] = np.eye(128, dtype=np.float32)
        bankT[0:128, BROW + 128] = 1.0
        bankT[0, BROW + 129:BROW + 257] = 1.0
        banks.append(np.ascontiguousarray(bankT).astype(bf16_np))

    xpad = np.zeros((B, D, PH_, PW), dtype=np.float32)
    xpad[:, :, 1:1 + H, 1:1 + W] = x.astype(np.float32).reshape(B, D, H, W)
    xpad = xpad.reshape(B, D, PH_ * PW).astype(bf16_np)

    breq = bank_request.astype(np.float32)
    sty = style.astype(np.float32).reshape(B, D)

    maps = []
    for c in range(B):
        oc = c % 2
        s0 = 2 * (c // 2)
        ctl = np.concatenate([breq[s0], breq[s0 + 1], sty[s0], sty[s0 + 1]])
        maps.append({
            "x": np.ascontiguousarray(xpad[s0:s0 + 2].reshape(2 * D, PH_ * PW)),
            "bank": banks[oc],
            "ctl": np.ascontiguousarray(ctl.reshape(1, CTL)),
        })
    return maps


def run(inputs, trace=False, **trace_kwargs):
    nc = _get_compiled()
    in_maps = _make_in_maps(inputs["x"], inputs["bank_request"],
                            inputs["style"], inputs["bank_weight"])
    # The first execution of a freshly compiled NEFF occasionally dies with
    # NRT_EXEC_UNIT_UNRECOVERABLE on this runtime; a plain retry succeeds.
    last_exc = None
    for _ in range(3):
        try:
            res = run_bass_kernel_spmd(nc, in_maps, core_ids=list(range(B)),
                                       trace=trace, **trace_kwargs)
            y = np.empty((B, D, H, W), dtype=np.float32)
            for c in range(B):
                oc = c % 2
                s0 = 2 * (c // 2)
                yc = res.results[c]["y"].reshape(2, 128, H, W)
                y[s0, oc * 128:(oc + 1) * 128] = yc[0]
                y[s0 + 1, oc * 128:(oc + 1) * 128] = yc[1]
            return y, res
        except Exception as e:  # noqa: BLE001
            last_exc = e
    raise last_exc


def kernel(x, bank_request, style, bank_weight):
    y, _ = run({"x": np.asarray(x), "bank_request": np.asarray(bank_request),
                "style": np.asarray(style), "bank_weight": np.asarray(bank_weight)})
    return y
